# revision 24
# baseline (speedup 1.0000x reference)
"""Trainium2 Bass kernel for nn_AttributeOperator (MoE-style routing).

Computes out[b] = relu(attr_ops[attrs[b]] @ obj_emb[objs[b]]) for b in [0, B).

Strategy (expert-parallel, fp8, column-tiled): the dominant cost is streaming
the attr_ops table (512 x 512 x 512 fp32 = 512 MB). Samples are grouped by
attribute on the host, groups are chunked to <= 32 samples, and chunks are
load-balanced across the 8 cores (snake deal by chunk size); each core streams
only its own subset of operator matrices from HBM exactly once, cast on the
host to fp8-e3m4 scaled by OPS_SCALE (the 1/OPS_SCALE dequant is folded into
the fp16 xt). e3m4's 4 mantissa bits give rel-err 1.39e-2 vs the f32
reference (gate 2e-2); e4m3 fails at 2.3e-2, which rules out DoubleRow.

Per core the work is `sg` super-groups of 4 slots; slot rank s has a fixed
capacity cap[s] = max chunk size at that rank over all cores (SPMD: one
program fits every core's routing). The four slots of a super-group run
CONCURRENTLY in the four 32-column groups of the PE array via
tile_position=(0, 32*j), all accumulating into one [128, 512] PSUM bank (slot
j at partitions 32*j..32*j+cap). This 4x-overlaps the A-matrix streaming (the
PE-time floor is the 1 col/cycle moving-operand ingest) and lets one
full-width [128, 512] relu + one 1 MB ops DMA serve 4 slots. Outputs are
relu'd in fp16 into a buffer spanning OW=4 super-groups, then written by 4
partition-sliced DMAs (col-group j rows only, <= cap of the window) so the
out stream is ~0.4 MB instead of 2 MB of padding. This keeps the serial
HWDGE descriptor-generation cost (~0.6 us per DMA), the ACT relu cost, and
the PE stream off the critical path; the ops stream owns the sync HWDGE ring
(xt/out use the scalar ring) and is the memory-roofline bottleneck
(~16.3 MB/core at ~380-440 GB/s effective).
"""

import numpy as np
import ml_dtypes

import concourse.tile as tile
from concourse import bacc, mybir
from concourse.bass_utils import run_bass_kernel_spmd

N_CORES = 8
D = 512               # embedding dim (hardcoded per problem spec)
QCH = D // 128        # contraction chunks of 128 partitions
CW = 32               # column-tile width = per-slot sample capacity
NJ = 4                # concurrent col-tiled slots per super-group
SG = 16               # minimum super-groups per core

# attr_ops values are ~N(0, 0.02); unscaled they'd land in e3m4's subnormal
# range, so scale into the normal range and fold 1/OPS_SCALE into xt.
OPS_SCALE = 64.0

# test.py hooks (ignored by the grading harness)
LAST_RESULTS = None   # BassKernelResults of the most recent run
TRACE = False
TRACE_CORES = None

OPG = 1               # super-groups per ops DMA
OW = 4                # super-groups per output window/buffer
_NC_CACHE = {}


def _build_nc(caps, ops_bufs=8, opg=None, reps=1, staggered=False):
    """Build + compile the SPMD program.

    caps[s]: sample capacity of slot rank s (s = g*NJ + j), len divisible by
    2*NJ; opg: super-groups per ops DMA; reps: HW-loop repetitions (timing).
    """
    if opg is None:
        opg = OPG
    sg = len(caps) // NJ
    ct = sum(caps)
    coff = np.concatenate([[0], np.cumsum(caps)])
    ng = -(-sg // opg)
    # output windows of OW super-groups sharing one o buffer; per window the
    # 4 col-groups are written by 4 partition-sliced DMAs of owcap rows each
    nw = -(-sg // OW)
    owcap = [max(caps[w * OW * NJ:(w + 1) * OW * NJ]) for w in range(nw)]
    owoff = np.concatenate([[0], np.cumsum([NJ * c for c in owcap])])
    tot = int(owoff[-1])
    nc = bacc.Bacc("TRN2", target_bir_lowering=False, debug=False,
                   num_devices=N_CORES)
    # ops[gg, p, (g', j, q, i)] = OPS_SCALE * A_{g,j}[i, q*128 + p] in e3m4:
    # one contiguous opg*NJ*QCH*D-byte run per partition per DMA
    ops_dram = nc.dram_tensor("ops_t", [ng, 128, opg * NJ * QCH * D],
                              mybir.dt.float8e3, kind="ExternalInput").ap()
    # xt[p, q*ct + coff[s] + c] = X_s[c, q*128 + p] / OPS_SCALE
    xt_dram = nc.dram_tensor("xt", [128, QCH * ct], mybir.dt.float16,
                             kind="ExternalInput").ap()
    # out[owoff[w] + j*owcap[w] + c, (g%OW)*D + i] for slot s=(g,j), c<cap[s]
    out_dram = nc.dram_tensor("out", [max(tot, 1), OW * D], mybir.dt.float16,
                              kind="ExternalOutput").ap()

    with tile.TileContext(nc) as tc:
        with (
            tc.tile_pool(name="xt", bufs=2) as xt_pool,
            tc.tile_pool(name="ops", bufs=ops_bufs) as ops_pool,
            tc.tile_pool(name="ps", bufs=8, space="PSUM") as ps_pool,
            tc.tile_pool(name="o", bufs=3) as o_pool,
        ):
            # nj_of[g]: live (cap > 0) slots in super-group g — zero caps are
            # a suffix, so the live slots of a group are always j < nj_of[g]
            nj_of = [sum(1 for j in range(NJ) if caps[g * NJ + j] > 0)
                     for g in range(sg)]
            last_g = max(g for g in range(sg) if nj_of[g] > 0)

            def body():
                xt_sb = xt_pool.tile([128, QCH * ct], mybir.dt.float16)
                nc.scalar.dma_start(xt_sb[:], xt_dram[:])

                o = None
                for g in range(sg):
                    if nj_of[g] == 0:
                        break
                    if g % opg == 0:
                        m = ops_pool.tile([128, opg * NJ * QCH * D],
                                          mybir.dt.float8e3, tag="m")
                        # trim the transfer to the live slots of the group(s)
                        nlive = sum(nj_of[g:g + opg])
                        wd = nlive * QCH * D
                        nc.sync.dma_start(m[:, :wd],
                                          ops_dram[g // opg][:, :wd])
                    ps = ps_pool.tile([128, D], mybir.dt.float32, tag="ps")
                    for q in range(QCH):
                        for j in range(nj_of[g]):
                            s = g * NJ + j
                            cw = caps[s]
                            lhsT = xt_sb[:, q * ct + coff[s]:
                                         q * ct + coff[s] + cw]
                            rhs = m[:, (((g % opg) * NJ + j) * QCH + q) * D:
                                    (((g % opg) * NJ + j) * QCH + q + 1) * D]
                            nc.tensor.matmul(ps[32 * j:32 * j + cw, :],
                                             lhsT, rhs,
                                             start=(q == 0),
                                             stop=(q == QCH - 1),
                                             tile_position=(0, 32 * j))
                    w = g // OW
                    if g % OW == 0:
                        o = o_pool.tile([128, OW * D], mybir.dt.float16,
                                        tag="o")
                    nc.scalar.activation(
                        o[:, (g % OW) * D:(g % OW + 1) * D], ps[:],
                        mybir.ActivationFunctionType.Relu)
                    if g % OW == OW - 1 or g == last_g:
                        cm = owcap[w]
                        for j in range(NJ):
                            r0 = int(owoff[w]) + j * cm
                            if cm > 0:
                                nc.scalar.dma_start(
                                    out_dram[r0:r0 + cm, :],
                                    o[32 * j:32 * j + cm, :])

            if reps == 1:
                body()
            else:
                with tc.For_i(0, reps, 1,
                              hint_engines=(mybir.EngineType.PE,),
                              staggered_reset=staggered):
                    body()

    nc.compile()
    return nc


def _route(attrs):
    """Group sample indices by attribute, chunk to <= CW, snake-balance
    across cores. Returns per-core slot lists of (attr_id, idx_array),
    sorted by descending chunk size."""
    order = np.argsort(attrs, kind="stable")
    sorted_attrs = attrs[order]
    uniq, starts, counts = np.unique(sorted_attrs, return_index=True,
                                     return_counts=True)
    chunks = []
    for a, st, c in zip(uniq, starts, counts):
        idx = order[st:st + c]
        for o in range(0, c, CW):
            chunks.append((int(a), idx[o:o + CW]))
    chunks.sort(key=lambda t: -len(t[1]))
    per_core = [[] for _ in range(N_CORES)]
    for i, ch in enumerate(chunks):
        r, pos = divmod(i, N_CORES)
        k = pos if r % 2 == 0 else N_CORES - 1 - pos
        per_core[k].append(ch)
    return per_core


def _quantize_ops(attr_ops, attr_ids):
    """e3m4-quantize A^T for the given attribute ids.

    Returns {attr_id: [QCH, 128, D] e3m4 array} with [q, p, i] =
    OPS_SCALE * A[i, q*128 + p]."""
    out = {}
    ids = np.asarray(sorted(attr_ids))
    for blk in range(0, len(ids), 32):
        b = ids[blk:blk + 32]
        at = np.ascontiguousarray(
            attr_ops[b].transpose(0, 2, 1)) * np.float32(OPS_SCALE)
        q8 = at.astype(ml_dtypes.float8_e3m4).reshape(-1, QCH, 128, D)
        for i, a in enumerate(b):
            out[int(a)] = q8[i]
    return out


def _layout(per_core):
    """Per-slot-rank capacities shared by all cores; cap 0 marks a dummy
    rank (no core has a chunk there) whose matrix stream is skipped. Ranks
    are size-sorted, so zero caps always form a suffix."""
    nslots = max(1, max(len(s) for s in per_core))
    sg = max(SG, -(-nslots // NJ))
    sg += sg % 2
    caps = [0] * (sg * NJ)
    for slots in per_core:
        for s, (_, idx) in enumerate(slots):
            caps[s] = max(caps[s], len(idx))
    return tuple(caps)


def _prepare(attrs, objs, attr_ops, obj_emb):
    """Route + build per-core device input maps."""
    per_core = _route(attrs)
    caps = _layout(per_core)
    sg = len(caps) // NJ
    ng = -(-sg // OPG)
    ct = sum(caps)
    coff = np.concatenate([[0], np.cumsum(caps)])

    rep = (obj_emb[objs] * np.float32(1.0 / OPS_SCALE)).astype(np.float16)
    q8 = _quantize_ops(attr_ops, {a for s in per_core for a, _ in s})
    in_maps = []
    for k in range(N_CORES):
        slots = per_core[k]
        ops_t = np.zeros((ng * OPG, 128, NJ, QCH, D), ml_dtypes.float8_e3m4)
        xt = np.zeros((128, QCH, ct), np.float16)
        for s, (a, idx) in enumerate(slots):
            g, j = divmod(s, NJ)
            ops_t[g, :, j] = q8[a].transpose(1, 0, 2)
            # xt[p, q, coff[s] + c] = rep[idx[c], q*128 + p]
            xt[:, :, coff[s]:coff[s] + len(idx)] = rep[idx].reshape(
                len(idx), QCH, 128).transpose(2, 1, 0)
        # device layout [gg, p, (g', j, q, i)]: opg consecutive super-groups
        # concatenated along the free dim
        ops_dev = np.ascontiguousarray(
            ops_t.reshape(ng, OPG, 128, NJ * QCH * D).transpose(0, 2, 1, 3)
        ).reshape(ng, 128, OPG * NJ * QCH * D)
        in_maps.append({"ops_t": ops_dev, "xt": xt.reshape(128, -1)})
    return per_core, caps, in_maps


def kernel(attrs, objs, attr_ops, obj_emb):
    global LAST_RESULTS
    attrs = np.asarray(attrs)
    objs = np.asarray(objs)
    attr_ops = np.asarray(attr_ops, dtype=np.float32)
    obj_emb = np.asarray(obj_emb, dtype=np.float32)
    B = attrs.shape[0]
    d = obj_emb.shape[1]
    assert d == D and attr_ops.shape[1:] == (D, D)

    per_core, caps, in_maps = _prepare(attrs, objs, attr_ops, obj_emb)

    nc = _NC_CACHE.get(caps)
    if nc is None:
        nc = _NC_CACHE[caps] = _build_nc(caps)

    res = run_bass_kernel_spmd(nc, in_maps, core_ids=list(range(N_CORES)),
                               trace=TRACE, trace_cores=TRACE_CORES)
    LAST_RESULTS = res

    sg = len(caps) // NJ
    nw = -(-sg // OW)
    owcap = [max(caps[w * OW * NJ:(w + 1) * OW * NJ]) for w in range(nw)]
    owoff = np.concatenate([[0], np.cumsum([NJ * c for c in owcap])])
    out = np.zeros((B, d), np.float32)
    for k in range(N_CORES):
        out_k = res.results[k]["out"]  # [tot, OW*D] fp16
        for s, (a, idx) in enumerate(per_core[k]):
            g, j = divmod(s, NJ)
            w = g // OW
            r0 = int(owoff[w]) + j * owcap[w]
            out[idx] = out_k[r0:r0 + len(idx),
                             (g % OW) * D:(g % OW) * D + d]
    return out


# revision 25
# speedup vs baseline: 1.0111x; 1.0111x over previous
"""Trainium2 Bass kernel for nn_AttributeOperator (MoE-style routing).

Computes out[b] = relu(attr_ops[attrs[b]] @ obj_emb[objs[b]]) for b in [0, B).

Strategy (expert-parallel, fp8, column-tiled): the dominant cost is streaming
the attr_ops table (512 x 512 x 512 fp32 = 512 MB). Samples are grouped by
attribute on the host, groups are chunked to <= 32 samples, and chunks are
load-balanced across the 8 cores (snake deal by chunk size); each core streams
only its own subset of operator matrices from HBM exactly once, cast on the
host to fp8-e3m4 scaled by OPS_SCALE (the 1/OPS_SCALE dequant is folded into
the fp16 xt). e3m4's 4 mantissa bits give rel-err 1.39e-2 vs the f32
reference (gate 2e-2); e4m3 fails at 2.3e-2, which rules out DoubleRow.

Per core the work is `sg` super-groups of 4 slots; slot rank s has a fixed
capacity cap[s] = max chunk size at that rank over all cores (SPMD: one
program fits every core's routing). The four slots of a super-group run
CONCURRENTLY in the four 32-column groups of the PE array via
tile_position=(0, 32*j), all accumulating into one [128, 512] PSUM bank (slot
j at partitions 32*j..32*j+cap). This 4x-overlaps the A-matrix streaming (the
PE-time floor is the 1 col/cycle moving-operand ingest) and lets one
full-width [128, 512] relu + one 1 MB ops DMA serve 4 slots. Outputs are
relu'd in fp16 into a buffer spanning OW=4 super-groups, then written by 4
partition-sliced DMAs (col-group j rows only, <= cap of the window) so the
out stream is ~0.4 MB instead of 2 MB of padding. This keeps the serial
HWDGE descriptor-generation cost (~0.6 us per DMA), the ACT relu cost, and
the PE stream off the critical path; the ops stream owns the sync HWDGE ring
(xt/out use the scalar ring) and is the memory-roofline bottleneck
(~16.1 MB/core at ~380-440 GB/s effective).
"""

import numpy as np
import ml_dtypes

import concourse.tile as tile
from concourse import bacc, mybir
from concourse.bass_utils import run_bass_kernel_spmd

N_CORES = 8
D = 512               # embedding dim (hardcoded per problem spec)
QCH = D // 128        # contraction chunks of 128 partitions
CW = 32               # column-tile width = per-slot sample capacity
NJ = 4                # concurrent col-tiled slots per super-group
SG = 16               # minimum super-groups per core

# attr_ops values are ~N(0, 0.02); unscaled they'd land in e3m4's subnormal
# range, so scale into the normal range and fold 1/OPS_SCALE into xt.
OPS_SCALE = 64.0

# test.py hooks (ignored by the grading harness)
LAST_RESULTS = None   # BassKernelResults of the most recent run
TRACE = False
TRACE_CORES = None

OPG = 1               # super-groups per ops DMA
OW = 4                # super-groups per output window/buffer
_NC_CACHE = {}


def _build_nc(caps, ops_bufs=8, opg=None, reps=1, staggered=False):
    """Build + compile the SPMD program.

    caps[s]: sample capacity of slot rank s (s = g*NJ + j), len divisible by
    2*NJ; opg: super-groups per ops DMA; reps: HW-loop repetitions (timing).
    """
    if opg is None:
        opg = OPG
    sg = len(caps) // NJ
    ct = sum(caps)
    coff = np.concatenate([[0], np.cumsum(caps)])
    ng = -(-sg // opg)
    # output windows of OW super-groups sharing one o buffer; per window the
    # 4 col-groups are written by 4 partition-sliced DMAs of owcap rows each
    nw = -(-sg // OW)
    owcap = [max(caps[w * OW * NJ:(w + 1) * OW * NJ]) for w in range(nw)]
    owoff = np.concatenate([[0], np.cumsum([NJ * c for c in owcap])])
    tot = int(owoff[-1])
    nc = bacc.Bacc("TRN2", target_bir_lowering=False, debug=False,
                   num_devices=N_CORES)
    # ops[gg, p, (g', j, q, i)] = OPS_SCALE * A_{g,j}[i, q*128 + p] in e3m4:
    # one contiguous opg*NJ*QCH*D-byte run per partition per DMA
    ops_dram = nc.dram_tensor("ops_t", [ng, 128, opg * NJ * QCH * D],
                              mybir.dt.float8e3, kind="ExternalInput").ap()
    # xt[p, q*ct + coff[s] + c] = X_s[c, q*128 + p] / OPS_SCALE
    xt_dram = nc.dram_tensor("xt", [128, QCH * ct], mybir.dt.float16,
                             kind="ExternalInput").ap()
    # out[owoff[w] + j*owcap[w] + c, (g%OW)*D + i] for slot s=(g,j), c<cap[s]
    out_dram = nc.dram_tensor("out", [max(tot, 1), OW * D], mybir.dt.float16,
                              kind="ExternalOutput").ap()

    with tile.TileContext(nc) as tc:
        with (
            tc.tile_pool(name="xt", bufs=2) as xt_pool,
            tc.tile_pool(name="ops", bufs=ops_bufs) as ops_pool,
            tc.tile_pool(name="ps", bufs=8, space="PSUM") as ps_pool,
            tc.tile_pool(name="o", bufs=3) as o_pool,
        ):
            # nj_of[g]: live (cap > 0) slots in super-group g — zero caps are
            # a suffix, so the live slots of a group are always j < nj_of[g]
            nj_of = [sum(1 for j in range(NJ) if caps[g * NJ + j] > 0)
                     for g in range(sg)]
            last_g = max(g for g in range(sg) if nj_of[g] > 0)

            def body():
                xt_sb = xt_pool.tile([128, QCH * ct], mybir.dt.float16)
                nc.scalar.dma_start(xt_sb[:], xt_dram[:])

                o = None
                for g in range(sg):
                    if nj_of[g] == 0:
                        break
                    if g % opg == 0:
                        m = ops_pool.tile([128, opg * NJ * QCH * D],
                                          mybir.dt.float8e3, tag="m")
                        # trim the transfer to the live slots of the group(s)
                        nlive = sum(nj_of[g:g + opg])
                        wd = nlive * QCH * D
                        nc.sync.dma_start(m[:, :wd],
                                          ops_dram[g // opg][:, :wd])
                    ps = ps_pool.tile([128, D], mybir.dt.float32, tag="ps")
                    for q in range(QCH):
                        for j in range(nj_of[g]):
                            s = g * NJ + j
                            cw = caps[s]
                            lhsT = xt_sb[:, q * ct + coff[s]:
                                         q * ct + coff[s] + cw]
                            rhs = m[:, (((g % opg) * NJ + j) * QCH + q) * D:
                                    (((g % opg) * NJ + j) * QCH + q + 1) * D]
                            nc.tensor.matmul(ps[32 * j:32 * j + cw, :],
                                             lhsT, rhs,
                                             start=(q == 0),
                                             stop=(q == QCH - 1),
                                             tile_position=(0, 32 * j))
                    w = g // OW
                    if g % OW == 0:
                        o = o_pool.tile([128, OW * D], mybir.dt.float16,
                                        tag="o")
                    nc.scalar.activation(
                        o[:, (g % OW) * D:(g % OW + 1) * D], ps[:],
                        mybir.ActivationFunctionType.Relu)
                    if g % OW == OW - 1 or g == last_g:
                        cm = owcap[w]
                        for j in range(NJ):
                            r0 = int(owoff[w]) + j * cm
                            if cm > 0:
                                nc.scalar.dma_start(
                                    out_dram[r0:r0 + cm, :],
                                    o[32 * j:32 * j + cm, :])

            if reps == 1:
                body()
            else:
                with tc.For_i(0, reps, 1,
                              hint_engines=(mybir.EngineType.PE,),
                              staggered_reset=staggered):
                    body()

    nc.compile()
    return nc


def _route(attrs):
    """Group sample indices by attribute, chunk to <= CW, snake-balance
    across cores. Returns per-core slot lists of (attr_id, idx_array),
    sorted by descending chunk size."""
    order = np.argsort(attrs, kind="stable")
    sorted_attrs = attrs[order]
    uniq, starts, counts = np.unique(sorted_attrs, return_index=True,
                                     return_counts=True)
    chunks = []
    for a, st, c in zip(uniq, starts, counts):
        idx = order[st:st + c]
        for o in range(0, c, CW):
            chunks.append((int(a), idx[o:o + CW]))
    chunks.sort(key=lambda t: -len(t[1]))
    per_core = [[] for _ in range(N_CORES)]
    for i, ch in enumerate(chunks):
        r, pos = divmod(i, N_CORES)
        k = pos if r % 2 == 0 else N_CORES - 1 - pos
        per_core[k].append(ch)
    return per_core


def _quantize_ops(attr_ops, attr_ids):
    """e3m4-quantize A^T for the given attribute ids.

    Returns {attr_id: [QCH, 128, D] e3m4 array} with [q, p, i] =
    OPS_SCALE * A[i, q*128 + p]."""
    out = {}
    ids = np.asarray(sorted(attr_ids))
    for blk in range(0, len(ids), 32):
        b = ids[blk:blk + 32]
        at = np.ascontiguousarray(
            attr_ops[b].transpose(0, 2, 1)) * np.float32(OPS_SCALE)
        q8 = at.astype(ml_dtypes.float8_e3m4).reshape(-1, QCH, 128, D)
        for i, a in enumerate(b):
            out[int(a)] = q8[i]
    return out


def _layout(per_core):
    """Per-slot-rank capacities shared by all cores; cap 0 marks a dummy
    rank (no core has a chunk there) whose matrix stream is skipped. Ranks
    are size-sorted, so zero caps always form a suffix."""
    nslots = max(1, max(len(s) for s in per_core))
    sg = max(SG, -(-nslots // NJ))
    sg += sg % 2
    caps = [0] * (sg * NJ)
    for slots in per_core:
        for s, (_, idx) in enumerate(slots):
            caps[s] = max(caps[s], len(idx))
    return tuple(caps)


def _prepare(attrs, objs, attr_ops, obj_emb):
    """Route + build per-core device input maps."""
    per_core = _route(attrs)
    caps = _layout(per_core)
    sg = len(caps) // NJ
    ng = -(-sg // OPG)
    ct = sum(caps)
    coff = np.concatenate([[0], np.cumsum(caps)])

    rep = (obj_emb[objs] * np.float32(1.0 / OPS_SCALE)).astype(np.float16)
    q8 = _quantize_ops(attr_ops, {a for s in per_core for a, _ in s})
    in_maps = []
    for k in range(N_CORES):
        slots = per_core[k]
        ops_t = np.zeros((ng * OPG, 128, NJ, QCH, D), ml_dtypes.float8_e3m4)
        xt = np.zeros((128, QCH, ct), np.float16)
        for s, (a, idx) in enumerate(slots):
            g, j = divmod(s, NJ)
            ops_t[g, :, j] = q8[a].transpose(1, 0, 2)
            # xt[p, q, coff[s] + c] = rep[idx[c], q*128 + p]
            xt[:, :, coff[s]:coff[s] + len(idx)] = rep[idx].reshape(
                len(idx), QCH, 128).transpose(2, 1, 0)
        # device layout [gg, p, (g', j, q, i)]: opg consecutive super-groups
        # concatenated along the free dim
        ops_dev = np.ascontiguousarray(
            ops_t.reshape(ng, OPG, 128, NJ * QCH * D).transpose(0, 2, 1, 3)
        ).reshape(ng, 128, OPG * NJ * QCH * D)
        in_maps.append({"ops_t": ops_dev, "xt": xt.reshape(128, -1)})
    return per_core, caps, in_maps


def kernel(attrs, objs, attr_ops, obj_emb):
    global LAST_RESULTS
    attrs = np.asarray(attrs)
    objs = np.asarray(objs)
    attr_ops = np.asarray(attr_ops, dtype=np.float32)
    obj_emb = np.asarray(obj_emb, dtype=np.float32)
    B = attrs.shape[0]
    d = obj_emb.shape[1]
    assert d == D and attr_ops.shape[1:] == (D, D)

    per_core, caps, in_maps = _prepare(attrs, objs, attr_ops, obj_emb)

    nc = _NC_CACHE.get(caps)
    if nc is None:
        nc = _NC_CACHE[caps] = _build_nc(caps)

    res = run_bass_kernel_spmd(nc, in_maps, core_ids=list(range(N_CORES)),
                               trace=TRACE, trace_cores=TRACE_CORES)
    LAST_RESULTS = res

    sg = len(caps) // NJ
    nw = -(-sg // OW)
    owcap = [max(caps[w * OW * NJ:(w + 1) * OW * NJ]) for w in range(nw)]
    owoff = np.concatenate([[0], np.cumsum([NJ * c for c in owcap])])
    out = np.zeros((B, d), np.float32)
    for k in range(N_CORES):
        out_k = res.results[k]["out"]  # [tot, OW*D] fp16
        for s, (a, idx) in enumerate(per_core[k]):
            g, j = divmod(s, NJ)
            w = g // OW
            r0 = int(owoff[w]) + j * owcap[w]
            out[idx] = out_k[r0:r0 + len(idx),
                             (g % OW) * D:(g % OW) * D + d]
    return out
